# revision 20
# baseline (speedup 1.0000x reference)
"""BitLinear v6: x-stationary / weight-streaming, fp8(e3m4) weights, 4-col-tile
PE packing, dual-ring DMA descriptor generation.

Orientation: out[b,o] = sum_k x[b,k] w[o,k] as 32 accumulating matmuls per
o-chunk with lhsT = xT_g [128k, 32b] (stationary, bf16) and rhs = W_g
[128k, 344] (moving, e3m4 prescaled sign*scale*64; /64 folded into the host
bf16 x image).  tile_position=(0,32j) packs 4 concurrent M=32 matmuls (one per
o-chunk) into the PE array; accumulation over g stays in one PSUM bank per
chunk so the drain is a plain copy (split vector/scalar halves).

DMA: the 5.63 MB/core e3m4 weight image goes in 6 slices with descriptor
generation alternating between the sync and scalar HWDGE rings (descriptor gen
costs ~700ns serialized per ring; v4 lost 12us to a single-ring chain).  Total
dma_starts (x + 6 w + y) = 8 = the Tile scheduler's DMAHW semaphore lanes.
Final slice is 1 group so the matmul trail after the last weight byte is short.
"""

import numpy as np

BATCH = 32
IN_F = 4096
OUT_F = 11008
GROUP = 128
N_GROUPS = IN_F // GROUP  # 32
N_CORES = 8
O_SHARD = OUT_F // N_CORES  # 1376
N_OTILE = 4
O_TILE = O_SHARD // N_OTILE  # 344
W_IMG_F = N_GROUPS * O_SHARD  # 44032 fp8 bytes per partition
SLICE_GS = [6, 8, 8, 6, 3, 1]  # groups per DMA slice
SCALE_NORM = 64.0

_nc_cache = []


def build_nc():
    import concourse.bacc as bacc
    import concourse.mybir as mybir
    import concourse.tile as tile

    f32 = mybir.dt.float32
    bf16 = mybir.dt.bfloat16
    fp8 = mybir.dt.float8e3

    nc = bacc.Bacc(None, target_bir_lowering=False)
    x_d = nc.dram_tensor("xT", [128, N_GROUPS * BATCH], bf16, kind="ExternalInput")
    w_d = nc.dram_tensor("wT", [128, W_IMG_F], fp8, kind="ExternalInput")
    y_d = nc.dram_tensor("y", [128, O_TILE], f32, kind="ExternalOutput")

    with tile.TileContext(nc) as tc:
        with tc.tile_pool(name="const", bufs=1) as const, tc.tile_pool(
            name="psum", bufs=1, space="PSUM"
        ) as psum:
            x_sb = const.tile([128, N_GROUPS, BATCH], bf16, tag="x_sb")
            w_sb = const.tile([128, N_GROUPS, O_SHARD], fp8, tag="w_sb")
            y_sb = const.tile([128, O_TILE], f32, tag="y_sb")
            dummy_sb = const.tile([128, 2 * O_SHARD], fp8, tag="dummy_sb")

            # x and all weight slices on ONE ring (sync) in consumption
            # order -- the scalar ring gets starved by a busy sync ring for
            # multiple us (arbitration), which can gate the first matmul.
            nc.sync.dma_start(
                x_sb[:], x_d[:].rearrange("p (g b) -> p g b", g=N_GROUPS)
            )
            g0 = 0
            for gs in SLICE_GS:
                nc.sync.dma_start(
                    w_sb[:, g0 : g0 + gs, :],
                    w_d[:, g0 * O_SHARD : (g0 + gs) * O_SHARD].rearrange(
                        "p (g o) -> p g o", g=gs
                    ),
                )
                g0 += gs
            # dummy trailer: keeps the SDMA pipeline primed so the LAST real
            # slice doesn't pay the ~1.5-2us end-of-queue crawl
            nc.sync.dma_start(dummy_sb[:], w_d[:, 0 : 2 * O_SHARD])

            ps = psum.tile([128, O_TILE], f32, tag="ps")
            for g in range(N_GROUPS):
                for j in range(N_OTILE):
                    nc.tensor.matmul(
                        ps[32 * j : 32 * (j + 1), :],
                        x_sb[:, g, :],
                        w_sb[:, g, j * O_TILE : (j + 1) * O_TILE],
                        start=(g == 0),
                        stop=(g == N_GROUPS - 1),
                        tile_position=(0, 32 * j),
                    )
            # single-engine drain (cross-engine split stalls on sem routing);
            # y DMA gen on sync, which is idle after the weight gens; a tiny
            # trailer after y keeps y off the end-of-queue crawl
            nc.vector.tensor_copy(y_sb[:], ps[:])
            nc.sync.dma_start(y_d[:], y_sb[:])
            nc.sync.dma_start(dummy_sb[:, :16], w_d[:, 0:16])
    nc.finalize()
    return nc


def _pack_weights(signs_shard, scales_shard):
    """[O_SHARD, IN_F] +/-1 int and [O_SHARD, N_GROUPS] f32 -> e3m4 image
    [128, W_IMG_F]: img[p, 1376*g + o] = sign[o, 128g+p]*scale[o,g]*64."""
    import ml_dtypes

    w = signs_shard.astype(np.float32) * np.repeat(
        scales_shard.astype(np.float32) * SCALE_NORM, GROUP, axis=1
    )  # [O_SHARD, IN_F]
    img = (
        w.T.reshape(N_GROUPS, GROUP, O_SHARD)
        .transpose(1, 0, 2)
        .reshape(128, W_IMG_F)
        .astype(ml_dtypes.float8_e3m4)
    )
    return img


def _pack_x(x):
    """[32, 4096] f32 -> bf16 image [128, N_GROUPS*BATCH]:
    img[p, 32*g + b] = x[b, 128g+p] / 64."""
    import ml_dtypes

    return np.ascontiguousarray(
        (x.astype(np.float32).T / SCALE_NORM)
        .reshape(N_GROUPS, GROUP, BATCH)
        .transpose(1, 0, 2)
        .reshape(128, N_GROUPS * BATCH)
        .astype(ml_dtypes.bfloat16)
    )


def _shard_inputs(x, scales, signs):
    scales_r = np.asarray(scales).reshape(OUT_F, N_GROUPS)
    signs = np.asarray(signs)
    x_img = _pack_x(np.asarray(x))
    in_maps = []
    for c in range(N_CORES):
        lo, hi = c * O_SHARD, (c + 1) * O_SHARD
        in_maps.append(
            {
                "xT": x_img,
                "wT": _pack_weights(signs[lo:hi], scales_r[lo:hi]),
            }
        )
    return in_maps


def _run(x, scales, signs, trace=False, tmpdir=None):
    from concourse import bass_utils

    if not _nc_cache:
        _nc_cache.append(build_nc())
    nc = _nc_cache[0]
    in_maps = _shard_inputs(x, scales, signs)
    res = bass_utils.run_bass_kernel_spmd(
        nc, in_maps, list(range(N_CORES)), trace=trace, tmpdir=tmpdir
    )
    # per core: y [128, 344] where partition 32j+b holds out[b, o_tile j]
    parts = []
    for i in range(N_CORES):
        yc = np.asarray(res.results[i]["y"]).reshape(N_OTILE, 32, O_TILE)
        parts.append(yc.transpose(1, 0, 2).reshape(BATCH, O_SHARD))
    out = np.concatenate(parts, axis=1)
    return np.ascontiguousarray(out).astype(np.float32), res


def kernel(x, scales, signs):
    out, _ = _run(x, scales, signs)
    return out


# revision 23
# speedup vs baseline: 1.0403x; 1.0403x over previous
"""BitLinear v6: x-stationary / weight-streaming, fp8(e3m4) weights, 4-col-tile
PE packing, dual-ring DMA descriptor generation.

Orientation: out[b,o] = sum_k x[b,k] w[o,k] as 32 accumulating matmuls per
o-chunk with lhsT = xT_g [128k, 32b] (stationary, bf16) and rhs = W_g
[128k, 344] (moving, e3m4 prescaled sign*scale*64; /64 folded into the host
bf16 x image).  tile_position=(0,32j) packs 4 concurrent M=32 matmuls (one per
o-chunk) into the PE array; accumulation over g stays in one PSUM bank per
chunk so the drain is a plain copy (split vector/scalar halves).

DMA: the 5.63 MB/core e3m4 weight image goes in 6 slices with descriptor
generation alternating between the sync and scalar HWDGE rings (descriptor gen
costs ~700ns serialized per ring; v4 lost 12us to a single-ring chain).  Total
dma_starts (x + 6 w + y) = 8 = the Tile scheduler's DMAHW semaphore lanes.
Final slice is 1 group so the matmul trail after the last weight byte is short.
"""

import numpy as np

BATCH = 32
IN_F = 4096
OUT_F = 11008
GROUP = 128
N_GROUPS = IN_F // GROUP  # 32
N_CORES = 8
O_SHARD = OUT_F // N_CORES  # 1376
N_OTILE = 4
O_TILE = O_SHARD // N_OTILE  # 344
W_IMG_F = N_GROUPS * O_SHARD  # 44032 fp8 bytes per partition
SLICE_GS = [6, 8, 8, 6, 3, 1]  # groups per DMA slice
SCALE_NORM = 64.0

_nc_cache = []


def build_nc():
    import concourse.bacc as bacc
    import concourse.mybir as mybir
    import concourse.tile as tile

    f32 = mybir.dt.float32
    bf16 = mybir.dt.bfloat16
    fp8 = mybir.dt.float8e3

    nc = bacc.Bacc(None, target_bir_lowering=False)
    x_d = nc.dram_tensor("xT", [128, N_GROUPS * BATCH], bf16, kind="ExternalInput")
    w_d = nc.dram_tensor("wT", [128, W_IMG_F], fp8, kind="ExternalInput")
    y_d = nc.dram_tensor("y", [128, O_TILE], f32, kind="ExternalOutput")

    with tile.TileContext(nc) as tc:
        with tc.tile_pool(name="const", bufs=1) as const, tc.tile_pool(
            name="psum", bufs=1, space="PSUM"
        ) as psum:
            x_sb = const.tile([128, N_GROUPS, BATCH], bf16, tag="x_sb")
            w_sb = const.tile([128, N_GROUPS, O_SHARD], fp8, tag="w_sb")
            y_sb = const.tile([128, O_TILE], f32, tag="y_sb")
            dummy_sb = const.tile([128, O_SHARD], fp8, tag="dummy_sb")

            # x and all weight slices on ONE ring (sync) in consumption
            # order -- the scalar ring gets starved by a busy sync ring for
            # multiple us (arbitration), which can gate the first matmul.
            nc.sync.dma_start(
                x_sb[:], x_d[:].rearrange("p (g b) -> p g b", g=N_GROUPS)
            )
            g0 = 0
            for gs in SLICE_GS:
                nc.sync.dma_start(
                    w_sb[:, g0 : g0 + gs, :],
                    w_d[:, g0 * O_SHARD : (g0 + gs) * O_SHARD].rearrange(
                        "p (g o) -> p g o", g=gs
                    ),
                )
                g0 += gs
            # dummy trailer: keeps the SDMA pipeline primed so the LAST real
            # slice doesn't pay the ~1.5-2us end-of-queue crawl
            nc.sync.dma_start(dummy_sb[:], w_d[:, 0:O_SHARD])

            ps = psum.tile([128, O_TILE], f32, tag="ps")
            for g in range(N_GROUPS):
                for j in range(N_OTILE):
                    nc.tensor.matmul(
                        ps[32 * j : 32 * (j + 1), :],
                        x_sb[:, g, :],
                        w_sb[:, g, j * O_TILE : (j + 1) * O_TILE],
                        start=(g == 0),
                        stop=(g == N_GROUPS - 1),
                        tile_position=(0, 32 * j),
                    )
            # single-engine drain (cross-engine split stalls on sem routing);
            # y DMA gen on sync, which is idle after the weight gens
            nc.vector.tensor_copy(y_sb[:], ps[:])
            nc.sync.dma_start(y_d[:], y_sb[:])
    nc.finalize()
    return nc


def _pack_weights(signs_shard, scales_shard):
    """[O_SHARD, IN_F] +/-1 int and [O_SHARD, N_GROUPS] f32 -> e3m4 image
    [128, W_IMG_F]: img[p, 1376*g + o] = sign[o, 128g+p]*scale[o,g]*64."""
    import ml_dtypes

    w = signs_shard.astype(np.float32) * np.repeat(
        scales_shard.astype(np.float32) * SCALE_NORM, GROUP, axis=1
    )  # [O_SHARD, IN_F]
    img = (
        w.T.reshape(N_GROUPS, GROUP, O_SHARD)
        .transpose(1, 0, 2)
        .reshape(128, W_IMG_F)
        .astype(ml_dtypes.float8_e3m4)
    )
    return img


def _pack_x(x):
    """[32, 4096] f32 -> bf16 image [128, N_GROUPS*BATCH]:
    img[p, 32*g + b] = x[b, 128g+p] / 64."""
    import ml_dtypes

    return np.ascontiguousarray(
        (x.astype(np.float32).T / SCALE_NORM)
        .reshape(N_GROUPS, GROUP, BATCH)
        .transpose(1, 0, 2)
        .reshape(128, N_GROUPS * BATCH)
        .astype(ml_dtypes.bfloat16)
    )


def _shard_inputs(x, scales, signs):
    scales_r = np.asarray(scales).reshape(OUT_F, N_GROUPS)
    signs = np.asarray(signs)
    x_img = _pack_x(np.asarray(x))
    in_maps = []
    for c in range(N_CORES):
        lo, hi = c * O_SHARD, (c + 1) * O_SHARD
        in_maps.append(
            {
                "xT": x_img,
                "wT": _pack_weights(signs[lo:hi], scales_r[lo:hi]),
            }
        )
    return in_maps


def _run(x, scales, signs, trace=False, tmpdir=None):
    from concourse import bass_utils

    if not _nc_cache:
        _nc_cache.append(build_nc())
    nc = _nc_cache[0]
    in_maps = _shard_inputs(x, scales, signs)
    res = bass_utils.run_bass_kernel_spmd(
        nc, in_maps, list(range(N_CORES)), trace=trace, tmpdir=tmpdir
    )
    # per core: y [128, 344] where partition 32j+b holds out[b, o_tile j]
    parts = []
    for i in range(N_CORES):
        yc = np.asarray(res.results[i]["y"]).reshape(N_OTILE, 32, O_TILE)
        parts.append(yc.transpose(1, 0, 2).reshape(BATCH, O_SHARD))
    out = np.concatenate(parts, axis=1)
    return np.ascontiguousarray(out).astype(np.float32), res


def kernel(x, scales, signs):
    out, _ = _run(x, scales, signs)
    return out
